# revision 19
# baseline (speedup 1.0000x reference)
"""Multi-head cross-attention (post-LN) Trainium2 Bass kernel.

Full inputs -> full outputs. Sharding: 8 cores = 4 batches x 2 query-row
halves (512 rows each).  Host pre-transposes h/c/weights so every matmul
contraction dim lands on SBUF partitions with no on-chip transposes.

Per-core pipeline (all matmuls float32r: full PE rate at free-dim 512):
  ph1: qT[e,i]  = WqT.T @ hT            (e on partitions, stays in SBUF)
  ph2: v[j,e]   = cT.T @ WvT            -> DRAM scratch (plain layout)
  ph3 per head pair (cT resident):
       kT_hp[e128,j] = WkT_pair.T @ cT  (fused, no spill)
       sT[j,i] = kT.T @ qT ; pT = exp(SCALE*sT)  (ACT, fused scale)
       avT[d,i] + denom row = v_aug.T @ pT       (PSUM accum over j,
           v_aug = per-pair V tiles re-loaded with a ones column)
       avT normalized by 1/denom (PE outer-product broadcast), kept in SBUF
  ph4: attn_out[i,o] = avT.T @ WoT ; out = LN(attn_out + h) * gamma + beta
"""

import sys

for _p in ("/opt/trn_rl_repo", "/root/.axon_site/_ro/trn_rl_repo"):
    if _p not in sys.path:
        sys.path.append(_p)

import numpy as np

import concourse.bass as bass
import concourse.tile as tile
from concourse import bacc, mybir
from concourse.bass_utils import run_bass_kernel_spmd

P = 128
D = 1024          # d_model
I = 512           # query rows per core
J = 2048          # kv length
NH = 16           # heads
DH = 64           # head dim
SCALE = 1.0 / (DH ** 0.5)
LN_EPS = 1e-5
F32 = mybir.dt.float32
F32R = mybir.dt.float32r

MT = D // P       # 8 m-tiles (contraction over d_model)
ET = D // P       # 8 e-tiles (head features)
JT = J // P       # 16 j-tiles
JB = J // 512     # 4 j-blocks of 512
NPAIR = NH // 2   # 8 head pairs
NCHUNK = 8        # score chunks per head: 2 j-tiles each (2 PSUM banks)


def build_program(reps=1):
    nc = bacc.Bacc(None, target_bir_lowering=False, debug=False)

    hT = nc.dram_tensor("hT", [D, I], F32R, kind="ExternalInput")
    cT = nc.dram_tensor("cT", [D, J], F32R, kind="ExternalInput")
    wqT = nc.dram_tensor("wqT", [D, D], F32R, kind="ExternalInput")
    wkT = nc.dram_tensor("wkT", [D, D], F32R, kind="ExternalInput")
    wvT = nc.dram_tensor("wvT", [D, D], F32R, kind="ExternalInput")
    woT = nc.dram_tensor("woT", [D, D], F32R, kind="ExternalInput")
    hres = nc.dram_tensor("hres", [I, D], F32, kind="ExternalInput")
    gamma = nc.dram_tensor("gamma", [P, D], F32, kind="ExternalInput")
    beta = nc.dram_tensor("beta", [P, D], F32, kind="ExternalInput")
    out = nc.dram_tensor("out", [I, D], F32, kind="ExternalOutput")

    with tile.TileContext(nc) as tc:
        with (
            tc.tile_pool(name="consts", bufs=1) as consts,
            tc.tile_pool(name="persist", bufs=1) as persist,
            tc.tile_pool(name="psum", bufs=1, space="PSUM") as psum,
            tc.tile_pool(name="dram", bufs=1, space="DRAM") as dram,
        ):
            # ---- constants & persistents ----------------------------------
            gamma_bc = consts.tile([P, D], F32, tag="gamma_bc")
            beta_bc = consts.tile([P, D], F32, tag="beta_bc")
            nc.sync.dma_start(gamma_bc, gamma.ap())
            nc.sync.dma_start(beta_bc, beta.ap())
            ones_row = consts.tile([1, DH], F32R, tag="ones_row")
            nc.vector.memset(ones_row.bitcast(F32), 1.0)
            eps_t = consts.tile([P, 1], F32, tag="eps")
            nc.vector.memset(eps_t, LN_EPS)

            qT = persist.tile([P, ET, I], F32R, tag="qT")       # 16KB/part
            avT = persist.tile([P, ET, I], F32R, tag="avT")     # 16KB/part
            cT_t = persist.tile([P, MT, J], F32R, tag="cT")     # 64KB/part
            v_dram = dram.tile([JT, P, D], F32R)

            for _rep in range(reps):
                nc.sync.dma_start(cT_t, cT.rearrange("(mt p) j -> p mt j", p=P))

                # ===== ph1: Q projection ===============================
                with tc.tile_pool(name="ph1", bufs=1) as ph1pool:
                    hT_t = ph1pool.tile([P, MT, I], F32R, tag="hT")
                    nc.sync.dma_start(hT_t, hT.rearrange("(mt p) i -> p mt i", p=P))
                    wq_t = []
                    for mt in range(MT):
                        w = ph1pool.tile([P, D], F32R, tag=f"wq{mt}")
                        nc.sync.dma_start(w, wqT.ap()[mt * P : (mt + 1) * P, :])
                        wq_t.append(w)
                    for et in range(ET):
                        ps = psum.tile([P, 512], F32, tag=("avA", "avB")[et % 2],
                                       name="q_ps")
                        for mt in range(MT):
                            nc.tensor.matmul(
                                ps,
                                wq_t[mt][:, et * P : (et + 1) * P],
                                hT_t[:, mt, :],
                                start=(mt == 0),
                                stop=(mt == MT - 1),
                            )
                        nc.vector.tensor_copy(qT[:, et, :], ps)

                # ===== ph2: V projection -> DRAM =======================
                with tc.tile_pool(name="ph2", bufs=1) as ph2pool:
                    for eh in range(2):
                        wv_t = []
                        for mt in range(MT):
                            w = ph2pool.tile([P, D // 2], F32R, tag=f"wv{mt}")
                            nc.sync.dma_start(
                                w, wvT.ap()[mt * P : (mt + 1) * P,
                                            eh * (D // 2) : (eh + 1) * (D // 2)]
                            )
                            wv_t.append(w)
                        for jt in range(JT):
                            ps = psum.tile([P, 512], F32,
                                           tag=("avA", "avB")[jt % 2], name="v_ps")
                            for mt in range(MT):
                                nc.tensor.matmul(
                                    ps,
                                    cT_t[:, mt, jt * P : (jt + 1) * P],
                                    wv_t[mt],
                                    start=(mt == 0),
                                    stop=(mt == MT - 1),
                                )
                            vs = ph2pool.tile([P, 512], F32R, tag="vstage",
                                              name="vstage", bufs=4)
                            nc.vector.tensor_copy(vs, ps)
                            nc.sync.dma_start(
                                v_dram[jt, :, eh * 512 : (eh + 1) * 512], vs
                            )

                # ===== ph3: attention per head pair ====================
                with tc.tile_pool(name="ph3", bufs=2) as ph3pool:
                    for hp in range(NPAIR):
                        # K^T for this pair, fused (no spill)
                        wk_t = []
                        for mt in range(MT):
                            w = ph3pool.tile([P, P], F32R, tag=f"wk{mt}",
                                             name="wk", bufs=2)
                            nc.sync.dma_start(
                                w, wkT.ap()[mt * P : (mt + 1) * P,
                                            hp * P : (hp + 1) * P]
                            )
                            wk_t.append(w)
                        kT_hp = ph3pool.tile([P, J], F32R, tag="kT_hp")
                        for jb in range(JB):
                            kps = psum.tile([P, 512], F32, tag="kps", name="kps")
                            for mt in range(MT):
                                nc.tensor.matmul(
                                    kps,
                                    wk_t[mt],
                                    cT_t[:, mt, jb * 512 : (jb + 1) * 512],
                                    start=(mt == 0),
                                    stop=(mt == MT - 1),
                                )
                            nc.vector.tensor_copy(
                                kT_hp[:, jb * 512 : (jb + 1) * 512], kps
                            )

                        # V tiles for this pair, ones-augmented
                        v_hp = ph3pool.tile([P, JT, 2, DH + 1], F32R, tag="v_hp")
                        nc.vector.memset(
                            v_hp[:, :, :, DH : DH + 1].bitcast(F32), 1.0
                        )
                        for h in range(2):
                            nc.sync.dma_start(
                                v_hp[:, :, h, 0:DH],
                                v_dram[:, :, (2 * hp + h) * DH :
                                       (2 * hp + h + 1) * DH]
                                .rearrange("jt p d -> p jt d"),
                            )

                        q_pair = qT[:, hp, :]
                        bounds = ((0, DH), (DH, P))
                        av_ps = [
                            psum.tile([P, I], F32, tag=("avA", "avB")[hi],
                                      name="avp")
                            for hi in range(2)
                        ]
                        for ci in range(NCHUNK):
                            scs = [
                                psum.tile([P, 2, 512], F32,
                                          tag=("scA", "scB")[hi], name="sc")
                                for hi in range(2)
                            ]
                            # interleave A/B so the K=64 matmuls row-pack on
                            # disjoint halves of the PE array
                            for k in range(2):
                                jt = 2 * ci + k
                                for hi, (p0, p1) in enumerate(bounds):
                                    nc.tensor.matmul(
                                        scs[hi][:, k, :],
                                        kT_hp[p0:p1, jt * P : (jt + 1) * P],
                                        q_pair[p0:p1, :],
                                        start=True,
                                        stop=True,
                                    )
                            pTs = []
                            for hi in range(2):
                                pT = ph3pool.tile([P, 2, 512], F32R,
                                                  tag=("pA", "pB")[hi], name="pT")
                                nc.scalar.activation(
                                    pT.rearrange("p a b -> p (a b)"),
                                    scs[hi].rearrange("p a b -> p (a b)"),
                                    mybir.ActivationFunctionType.Exp,
                                    scale=SCALE,
                                )
                                pTs.append(pT)
                            for k in range(2):
                                jt = 2 * ci + k
                                for hi in range(2):
                                    nc.tensor.matmul(
                                        av_ps[hi][0 : DH + 1, :],
                                        v_hp[:, jt, hi, :],
                                        pTs[hi][:, k, :],
                                        start=(jt == 0),
                                        stop=(jt == JT - 1),
                                    )

                        for hi in range(2):
                            recip = ph3pool.tile([1, I], F32R, tag="recip",
                                                 name="recip")
                            with nc.allow_low_precision(
                                reason="f32r keeps the f32 mantissa in SBUF"
                            ):
                                nc.vector.reciprocal(
                                    recip, av_ps[hi][DH : DH + 1, :]
                                )
                            # replicate [1, I] across DH partitions on the PE
                            rbc_ps = psum.tile([DH, I], F32,
                                               tag=("scA", "scB")[hi],
                                               name="rbc_ps")
                            nc.tensor.matmul(rbc_ps, ones_row, recip,
                                             start=True, stop=True)
                            rbc = ph3pool.tile([DH, I], F32, tag="rbc",
                                               name="rbc")
                            nc.vector.tensor_copy(rbc, rbc_ps)
                            nc.vector.tensor_tensor(
                                avT[hi * DH : (hi + 1) * DH, hp, :],
                                av_ps[hi][0:DH, :],
                                rbc,
                                mybir.AluOpType.mult,
                            )

                # ===== ph4: out-proj + residual + LN ===================
                with (
                    tc.tile_pool(name="ph4w", bufs=1) as ph4w,
                    tc.tile_pool(name="ph4", bufs=2) as ph4pool,
                ):
                    wo_t = []
                    for et in range(ET):
                        w = ph4w.tile([P, D], F32R, tag=f"wo{et}")
                        nc.sync.dma_start(w, woT.ap()[et * P : (et + 1) * P, :])
                        wo_t.append(w)
                    for it in range(I // P):
                        po = psum.tile([P, 2, 512], F32,
                                       tag=("scA", "scB")[it % 2], name="po")
                        for ob in range(2):
                            for et in range(ET):
                                nc.tensor.matmul(
                                    po[:, ob, :],
                                    avT[:, et, it * P : (it + 1) * P],
                                    wo_t[et][:, ob * 512 : (ob + 1) * 512],
                                    start=(et == 0),
                                    stop=(et == ET - 1),
                                )
                        hres_t = ph4pool.tile([P, D], F32, tag="hres")
                        nc.sync.dma_start(hres_t,
                                          hres.ap()[it * P : (it + 1) * P, :])
                        x = ph4pool.tile([P, D], F32, tag="x")
                        nc.vector.tensor_tensor(
                            x, po.rearrange("p a b -> p (a b)"), hres_t,
                            mybir.AluOpType.add,
                        )
                        stats = ph4pool.tile([P, 2, nc.vector.BN_STATS_DIM], F32,
                                             tag="stats")
                        xg = x.rearrange("p (g d) -> p g d", g=2)
                        for g in range(2):
                            nc.vector.bn_stats(stats[:, g, :], xg[:, g, :])
                        mv = ph4pool.tile([P, nc.vector.BN_AGGR_DIM], F32,
                                          tag="mv")
                        nc.vector.bn_aggr(mv, stats)
                        rstd = ph4pool.tile([P, 1], F32, tag="rstd")
                        nc.scalar.activation(
                            rstd, mv[:, 1:2], mybir.ActivationFunctionType.Sqrt,
                            bias=eps_t,
                        )
                        nc.vector.reciprocal(rstd, rstd)
                        nc.vector.tensor_scalar(
                            x, x, mv[:, 0:1], rstd,
                            op0=mybir.AluOpType.subtract,
                            op1=mybir.AluOpType.mult,
                        )
                        nc.vector.tensor_tensor(x, x, gamma_bc,
                                                mybir.AluOpType.mult)
                        nc.vector.tensor_tensor(x, x, beta_bc,
                                                mybir.AluOpType.add)
                        nc.sync.dma_start(out.ap()[it * P : (it + 1) * P, :], x)

    nc.compile()
    return nc


_NC_CACHE = {}


def _get_program(reps=1):
    if reps not in _NC_CACHE:
        _NC_CACHE[reps] = build_program(reps)
    return _NC_CACHE[reps]


def _make_in_maps(h, c, Wq, Wkv, Wo, gamma, beta):
    h = np.asarray(h, dtype=np.float32)
    c = np.asarray(c, dtype=np.float32)
    Wq = np.asarray(Wq, dtype=np.float32)
    Wkv = np.asarray(Wkv, dtype=np.float32)
    Wo = np.asarray(Wo, dtype=np.float32)
    gamma = np.asarray(gamma, dtype=np.float32)
    beta = np.asarray(beta, dtype=np.float32)

    q_len, batch, d_model = h.shape
    assert (q_len, batch, d_model) == (1024, 4, D)

    wqT = np.ascontiguousarray(Wq.T)
    wkT = np.ascontiguousarray(Wkv[:D].T)
    wvT = np.ascontiguousarray(Wkv[D:].T)
    woT = np.ascontiguousarray(Wo.T)
    gamma_b = np.ascontiguousarray(np.broadcast_to(gamma, (P, D)))
    beta_b = np.ascontiguousarray(np.broadcast_to(beta, (P, D)))

    in_maps = []
    for core in range(8):
        b, g = divmod(core, 2)
        i0, i1 = g * I, (g + 1) * I
        in_maps.append({
            "hT": np.ascontiguousarray(h[i0:i1, b, :].T),
            "cT": np.ascontiguousarray(c[:, b, :].T),
            "wqT": wqT,
            "wkT": wkT,
            "wvT": wvT,
            "woT": woT,
            "hres": np.ascontiguousarray(h[i0:i1, b, :]),
            "gamma": gamma_b,
            "beta": beta_b,
        })
    return in_maps


def kernel(h, c, Wq, Wkv, Wo, gamma, beta):
    in_maps = _make_in_maps(h, c, Wq, Wkv, Wo, gamma, beta)
    nc = _get_program()
    res = run_bass_kernel_spmd(nc, in_maps, core_ids=list(range(8)))

    q_len, batch = 1024, 4
    out = np.empty((q_len, batch, D), dtype=np.float32)
    for core in range(8):
        b, g = divmod(core, 2)
        out[g * I : (g + 1) * I, b, :] = res.results[core]["out"]
    return out


def bench(inputs, iters=20, reps=1, chain=8):
    """Time the on-device execution: warm jit + pre-transferred inputs,
    chained-dispatch slope (cancels per-call overhead)."""
    import time

    import jax
    from jax.experimental.shard_map import shard_map
    from jax.sharding import Mesh, NamedSharding, PartitionSpec

    from concourse import bass2jax, mybir as _mybir

    bass2jax.install_neuronx_cc_hook()
    nc = _get_program(reps)
    in_maps = _make_in_maps(**inputs)

    partition_name = nc.partition_id_tensor.name if nc.partition_id_tensor else None
    in_names, out_names, out_avals, zero_outs = [], [], [], []
    for alloc in nc.m.functions[0].allocations:
        if not isinstance(alloc, _mybir.MemoryLocationSet):
            continue
        name = alloc.memorylocations[0].name
        if alloc.kind == "ExternalInput":
            if name != partition_name:
                in_names.append(name)
        elif alloc.kind == "ExternalOutput":
            shape = tuple(alloc.tensor_shape)
            dtype = _mybir.dt.np(alloc.dtype)
            out_names.append(name)
            out_avals.append(jax.core.ShapedArray(shape, dtype))
            zero_outs.append(np.zeros(shape, dtype))
    n_params = len(in_names)
    all_in_names = list(in_names) + list(out_names)
    if partition_name is not None:
        all_in_names.append(partition_name)

    def _body(*args):
        operands = list(args)
        if partition_name is not None:
            operands.append(bass2jax.partition_id_tensor())
        outs = bass2jax._bass_exec_p.bind(
            *operands,
            out_avals=tuple(out_avals),
            in_names=tuple(all_in_names),
            out_names=tuple(out_names),
            lowering_input_output_aliases=(),
            sim_require_finite=True,
            sim_require_nnan=True,
            nc=nc,
        )
        return tuple(outs)

    n_outs = len(out_avals)
    donate = tuple(range(n_params, n_params + n_outs))
    devices = jax.devices()[:8]
    mesh = Mesh(np.asarray(devices), ("core",))
    in_specs = (PartitionSpec("core"),) * (n_params + n_outs)
    out_specs = (PartitionSpec("core"),) * n_outs
    sharded = jax.jit(
        shard_map(_body, mesh=mesh, in_specs=in_specs, out_specs=out_specs,
                  check_rep=False),
        donate_argnums=donate, keep_unused=True,
    )
    concat_in = [
        np.concatenate([np.asarray(in_maps[c][nm]) for c in range(8)], axis=0)
        for nm in in_names
    ]
    sh = NamedSharding(mesh, PartitionSpec("core"))
    dev_in = [jax.device_put(x, sh) for x in concat_in]

    def fresh_zeros():
        return [
            jax.device_put(np.zeros((8 * z.shape[0], *z.shape[1:]), z.dtype), sh)
            for z in zero_outs
        ]

    out = sharded(*dev_in, *fresh_zeros())
    jax.block_until_ready(out)

    def run_chain(k):
        zsets = [fresh_zeros() for _ in range(k)]
        for zs in zsets:
            jax.block_until_ready(zs)
        t0 = time.perf_counter()
        outs = [sharded(*dev_in, *zs) for zs in zsets]
        jax.block_until_ready(outs)
        return time.perf_counter() - t0

    run_chain(2)  # extra warmup
    slopes = []
    for _ in range(max(3, iters // 4)):
        t_a = run_chain(1)
        t_b = run_chain(chain)
        slopes.append((t_b - t_a) / (chain - 1.0))
    slopes.sort()
    med = slopes[len(slopes) // 2]
    print(f"bench(reps={reps}): slopes(us) = "
          f"{[f'{s*1e6:.0f}' for s in slopes]} -> median {med*1e6:.0f}us")
    return med * 1e9


# revision 23
# speedup vs baseline: 1.9519x; 1.9519x over previous
"""Multi-head cross-attention (post-LN) Trainium2 Bass kernel.

Full inputs -> full outputs. Sharding: 8 cores = 4 batches x 2 query-row
halves (512 rows each).  Host pre-transposes h/c/weights so every matmul
contraction dim lands on SBUF partitions with no on-chip transposes.

Per-core pipeline (all matmuls float32r: full PE rate at free-dim 512):
  ph1: qT[e,i]  = WqT.T @ hT            (e on partitions, stays in SBUF)
  ph2: v[j,e]   = cT.T @ WvT            -> DRAM scratch (plain layout)
  ph3 per head pair (cT resident):
       kT_hp[e128,j] = WkT_pair.T @ cT  (fused, no spill)
       sT[j,i] = kT.T @ qT ; pT = exp(SCALE*sT)  (ACT, fused scale)
       avT[d,i] + denom row = v_aug.T @ pT       (PSUM accum over j,
           v_aug = per-pair V tiles re-loaded with a ones column)
       avT normalized by 1/denom (PE outer-product broadcast), kept in SBUF
  ph4: attn_out[i,o] = avT.T @ WoT ; out = LN(attn_out + h) * gamma + beta
"""

import sys

for _p in ("/opt/trn_rl_repo", "/root/.axon_site/_ro/trn_rl_repo"):
    if _p not in sys.path:
        sys.path.append(_p)

import numpy as np

import concourse.bass as bass
import concourse.tile as tile
from concourse import bacc, mybir
from concourse.bass_utils import run_bass_kernel_spmd

P = 128
D = 1024          # d_model
I = 512           # query rows per core
J = 2048          # kv length
NH = 16           # heads
DH = 64           # head dim
SCALE = 1.0 / (DH ** 0.5)
LN_EPS = 1e-5
F32 = mybir.dt.float32
F32R = mybir.dt.float32r

MT = D // P       # 8 m-tiles (contraction over d_model)
ET = D // P       # 8 e-tiles (head features)
JT = J // P       # 16 j-tiles
JB = J // 512     # 4 j-blocks of 512
NPAIR = NH // 2   # 8 head pairs
NCHUNK = 8        # score chunks per head: 2 j-tiles each (2 PSUM banks)


def build_program(reps=1):
    nc = bacc.Bacc(None, target_bir_lowering=False, debug=False)

    hT = nc.dram_tensor("hT", [D, I], F32R, kind="ExternalInput")
    cT = nc.dram_tensor("cT", [D, J], F32R, kind="ExternalInput")
    wqT = nc.dram_tensor("wqT", [D, D], F32R, kind="ExternalInput")
    wkT = nc.dram_tensor("wkT", [D, D], F32R, kind="ExternalInput")
    wvT = nc.dram_tensor("wvT", [D, D], F32R, kind="ExternalInput")
    woT = nc.dram_tensor("woT", [D, D], F32R, kind="ExternalInput")
    hres = nc.dram_tensor("hres", [I, D], F32, kind="ExternalInput")
    gamma = nc.dram_tensor("gamma", [P, D], F32, kind="ExternalInput")
    beta = nc.dram_tensor("beta", [P, D], F32, kind="ExternalInput")
    out = nc.dram_tensor("out", [I, D], F32, kind="ExternalOutput")

    with tile.TileContext(nc) as tc:
        with (
            tc.tile_pool(name="consts", bufs=1) as consts,
            tc.tile_pool(name="persist", bufs=1) as persist,
            tc.tile_pool(name="psum", bufs=1, space="PSUM") as psum,
            tc.tile_pool(name="dram", bufs=1, space="DRAM") as dram,
        ):
            # ---- constants & persistents ----------------------------------
            gamma_bc = consts.tile([P, D], F32, tag="gamma_bc")
            beta_bc = consts.tile([P, D], F32, tag="beta_bc")
            nc.sync.dma_start(gamma_bc, gamma.ap())
            nc.sync.dma_start(beta_bc, beta.ap())
            ones_row = consts.tile([1, DH], F32R, tag="ones_row")
            nc.vector.memset(ones_row.bitcast(F32), 1.0)
            eps_t = consts.tile([P, 1], F32, tag="eps")
            nc.vector.memset(eps_t, LN_EPS)

            qT = persist.tile([P, ET, I], F32R, tag="qT")       # 16KB/part
            avT = persist.tile([P, ET, I], F32R, tag="avT")     # 16KB/part
            cT_t = persist.tile([P, MT, J], F32R, tag="cT")     # 64KB/part
            v_dram = dram.tile([JT, P, D], F32R)

            for _rep in range(reps):
                for mt in range(MT):  # split across DMA queues
                    nc.sync.dma_start(
                        cT_t[:, mt, :], cT.ap()[mt * P : (mt + 1) * P, :]
                    )

                # ===== ph1: Q projection ===============================
                with tc.tile_pool(name="ph1", bufs=1) as ph1pool:
                    hT_t = ph1pool.tile([P, MT, I], F32R, tag="hT")
                    for mt in range(MT):
                        nc.sync.dma_start(
                            hT_t[:, mt, :], hT.ap()[mt * P : (mt + 1) * P, :]
                        )
                    wq_t = []
                    for mt in range(MT):
                        w = ph1pool.tile([P, D], F32R, tag=f"wq{mt}")
                        nc.sync.dma_start(w, wqT.ap()[mt * P : (mt + 1) * P, :])
                        wq_t.append(w)
                    for et in range(ET):
                        ps = psum.tile([P, 512], F32, tag=("avA", "avB")[et % 2],
                                       name="q_ps")
                        for mt in range(MT):
                            nc.tensor.matmul(
                                ps,
                                wq_t[mt][:, et * P : (et + 1) * P],
                                hT_t[:, mt, :],
                                start=(mt == 0),
                                stop=(mt == MT - 1),
                            )
                        nc.vector.tensor_copy(qT[:, et, :], ps)

                # ===== ph2: V projection -> DRAM =======================
                with tc.tile_pool(name="ph2", bufs=1) as ph2pool:
                    for eh in range(2):
                        wv_t = []
                        for mt in range(MT):
                            w = ph2pool.tile([P, D // 2], F32R, tag=f"wv{mt}")
                            nc.sync.dma_start(
                                w, wvT.ap()[mt * P : (mt + 1) * P,
                                            eh * (D // 2) : (eh + 1) * (D // 2)]
                            )
                            wv_t.append(w)
                        for jt in range(JT):
                            ps = psum.tile([P, 512], F32,
                                           tag=("avA", "avB")[jt % 2], name="v_ps")
                            for mt in range(MT):
                                nc.tensor.matmul(
                                    ps,
                                    cT_t[:, mt, jt * P : (jt + 1) * P],
                                    wv_t[mt],
                                    start=(mt == 0),
                                    stop=(mt == MT - 1),
                                )
                            vs = ph2pool.tile([P, 512], F32R, tag="vstage",
                                              name="vstage", bufs=4)
                            nc.vector.tensor_copy(vs, ps)
                            nc.sync.dma_start(
                                v_dram[jt, :, eh * 512 : (eh + 1) * 512], vs
                            )

                # ===== ph3: attention per head pair ====================
                with tc.tile_pool(name="ph3", bufs=2) as ph3pool:
                    for hp in range(NPAIR):
                        # K^T for this pair, fused (no spill)
                        wk_t = []
                        for mt in range(MT):
                            w = ph3pool.tile([P, P], F32R, tag=f"wk{mt}",
                                             name="wk", bufs=2)
                            nc.sync.dma_start(
                                w, wkT.ap()[mt * P : (mt + 1) * P,
                                            hp * P : (hp + 1) * P]
                            )
                            wk_t.append(w)
                        kT_hp = ph3pool.tile([P, J], F32R, tag="kT_hp")
                        for jb in range(JB):
                            kps = psum.tile([P, 512], F32, tag="kps", name="kps")
                            for mt in range(MT):
                                nc.tensor.matmul(
                                    kps,
                                    wk_t[mt],
                                    cT_t[:, mt, jb * 512 : (jb + 1) * 512],
                                    start=(mt == 0),
                                    stop=(mt == MT - 1),
                                )
                            nc.vector.tensor_copy(
                                kT_hp[:, jb * 512 : (jb + 1) * 512], kps
                            )

                        # V tiles for this pair, ones-augmented
                        v_hp = ph3pool.tile([P, JT, 2, DH + 1], F32R, tag="v_hp")
                        nc.vector.memset(
                            v_hp[:, :, :, DH : DH + 1].bitcast(F32), 1.0
                        )
                        for h in range(2):
                            nc.sync.dma_start(
                                v_hp[:, :, h, 0:DH],
                                v_dram[:, :, (2 * hp + h) * DH :
                                       (2 * hp + h + 1) * DH]
                                .rearrange("jt p d -> p jt d"),
                            )

                        q_pair = qT[:, hp, :]
                        bounds = ((0, DH), (DH, P))
                        av_ps = [
                            psum.tile([P, I], F32, tag=("avA", "avB")[hi],
                                      name="avp")
                            for hi in range(2)
                        ]
                        for ci in range(NCHUNK):
                            scs = [
                                psum.tile([P, 2, 512], F32,
                                          tag=("scA", "scB")[hi], name="sc")
                                for hi in range(2)
                            ]
                            # interleave A/B so the K=64 matmuls row-pack on
                            # disjoint halves of the PE array
                            for k in range(2):
                                jt = 2 * ci + k
                                for hi, (p0, p1) in enumerate(bounds):
                                    nc.tensor.matmul(
                                        scs[hi][:, k, :],
                                        kT_hp[p0:p1, jt * P : (jt + 1) * P],
                                        q_pair[p0:p1, :],
                                        start=True,
                                        stop=True,
                                    )
                            pTs = []
                            for hi in range(2):
                                pT = ph3pool.tile([P, 2, 512], F32R,
                                                  tag=("pA", "pB")[hi], name="pT")
                                nc.scalar.activation(
                                    pT.rearrange("p a b -> p (a b)"),
                                    scs[hi].rearrange("p a b -> p (a b)"),
                                    mybir.ActivationFunctionType.Exp,
                                    scale=SCALE,
                                )
                                pTs.append(pT)
                            for k in range(2):
                                jt = 2 * ci + k
                                for hi in range(2):
                                    nc.tensor.matmul(
                                        av_ps[hi][0 : DH + 1, :],
                                        v_hp[:, jt, hi, :],
                                        pTs[hi][:, k, :],
                                        start=(jt == 0),
                                        stop=(jt == JT - 1),
                                    )

                        for hi in range(2):
                            recip = ph3pool.tile([1, I], F32R, tag="recip",
                                                 name="recip")
                            with nc.allow_low_precision(
                                reason="f32r keeps the f32 mantissa in SBUF"
                            ):
                                nc.vector.reciprocal(
                                    recip, av_ps[hi][DH : DH + 1, :]
                                )
                            # replicate [1, I] across DH partitions on the PE
                            rbc_ps = psum.tile([DH, I], F32,
                                               tag=("scA", "scB")[hi],
                                               name="rbc_ps")
                            nc.tensor.matmul(rbc_ps, ones_row, recip,
                                             start=True, stop=True)
                            rbc = ph3pool.tile([DH, I], F32, tag="rbc",
                                               name="rbc")
                            nc.vector.tensor_copy(rbc, rbc_ps)
                            nc.vector.tensor_tensor(
                                avT[hi * DH : (hi + 1) * DH, hp, :],
                                av_ps[hi][0:DH, :],
                                rbc,
                                mybir.AluOpType.mult,
                            )

                # ===== ph4: out-proj + residual + LN ===================
                with (
                    tc.tile_pool(name="ph4w", bufs=1) as ph4w,
                    tc.tile_pool(name="ph4", bufs=2) as ph4pool,
                ):
                    wo_t = []
                    for et in range(ET):
                        w = ph4w.tile([P, D], F32R, tag=f"wo{et}")
                        nc.sync.dma_start(w, woT.ap()[et * P : (et + 1) * P, :])
                        wo_t.append(w)
                    for it in range(I // P):
                        po = psum.tile([P, 2, 512], F32,
                                       tag=("scA", "scB")[it % 2], name="po")
                        for ob in range(2):
                            for et in range(ET):
                                nc.tensor.matmul(
                                    po[:, ob, :],
                                    avT[:, et, it * P : (it + 1) * P],
                                    wo_t[et][:, ob * 512 : (ob + 1) * 512],
                                    start=(et == 0),
                                    stop=(et == ET - 1),
                                )
                        hres_t = ph4pool.tile([P, D], F32, tag="hres")
                        nc.sync.dma_start(hres_t,
                                          hres.ap()[it * P : (it + 1) * P, :])
                        x = ph4pool.tile([P, D], F32, tag="x")
                        nc.vector.tensor_tensor(
                            x, po.rearrange("p a b -> p (a b)"), hres_t,
                            mybir.AluOpType.add,
                        )
                        stats = ph4pool.tile([P, 2, nc.vector.BN_STATS_DIM], F32,
                                             tag="stats")
                        xg = x.rearrange("p (g d) -> p g d", g=2)
                        for g in range(2):
                            nc.vector.bn_stats(stats[:, g, :], xg[:, g, :])
                        mv = ph4pool.tile([P, nc.vector.BN_AGGR_DIM], F32,
                                          tag="mv")
                        nc.vector.bn_aggr(mv, stats)
                        rstd = ph4pool.tile([P, 1], F32, tag="rstd")
                        nc.scalar.activation(
                            rstd, mv[:, 1:2], mybir.ActivationFunctionType.Sqrt,
                            bias=eps_t,
                        )
                        nc.vector.reciprocal(rstd, rstd)
                        nc.vector.tensor_scalar(
                            x, x, mv[:, 0:1], rstd,
                            op0=mybir.AluOpType.subtract,
                            op1=mybir.AluOpType.mult,
                        )
                        nc.vector.tensor_tensor(x, x, gamma_bc,
                                                mybir.AluOpType.mult)
                        nc.vector.tensor_tensor(x, x, beta_bc,
                                                mybir.AluOpType.add)
                        nc.sync.dma_start(out.ap()[it * P : (it + 1) * P, :], x)

    nc.compile()
    return nc


_NC_CACHE = {}


def _get_program(reps=1):
    if reps not in _NC_CACHE:
        _NC_CACHE[reps] = build_program(reps)
    return _NC_CACHE[reps]


def _make_in_maps(h, c, Wq, Wkv, Wo, gamma, beta):
    h = np.asarray(h, dtype=np.float32)
    c = np.asarray(c, dtype=np.float32)
    Wq = np.asarray(Wq, dtype=np.float32)
    Wkv = np.asarray(Wkv, dtype=np.float32)
    Wo = np.asarray(Wo, dtype=np.float32)
    gamma = np.asarray(gamma, dtype=np.float32)
    beta = np.asarray(beta, dtype=np.float32)

    q_len, batch, d_model = h.shape
    assert (q_len, batch, d_model) == (1024, 4, D)

    wqT = np.ascontiguousarray(Wq.T)
    wkT = np.ascontiguousarray(Wkv[:D].T)
    wvT = np.ascontiguousarray(Wkv[D:].T)
    woT = np.ascontiguousarray(Wo.T)
    gamma_b = np.ascontiguousarray(np.broadcast_to(gamma, (P, D)))
    beta_b = np.ascontiguousarray(np.broadcast_to(beta, (P, D)))

    in_maps = []
    for core in range(8):
        b, g = divmod(core, 2)
        i0, i1 = g * I, (g + 1) * I
        in_maps.append({
            "hT": np.ascontiguousarray(h[i0:i1, b, :].T),
            "cT": np.ascontiguousarray(c[:, b, :].T),
            "wqT": wqT,
            "wkT": wkT,
            "wvT": wvT,
            "woT": woT,
            "hres": np.ascontiguousarray(h[i0:i1, b, :]),
            "gamma": gamma_b,
            "beta": beta_b,
        })
    return in_maps


def kernel(h, c, Wq, Wkv, Wo, gamma, beta):
    in_maps = _make_in_maps(h, c, Wq, Wkv, Wo, gamma, beta)
    nc = _get_program()
    res = run_bass_kernel_spmd(nc, in_maps, core_ids=list(range(8)))

    q_len, batch = 1024, 4
    out = np.empty((q_len, batch, D), dtype=np.float32)
    for core in range(8):
        b, g = divmod(core, 2)
        out[g * I : (g + 1) * I, b, :] = res.results[core]["out"]
    return out


def bench(inputs, iters=20, reps=1, chain=8):
    """Time the on-device execution: warm jit + pre-transferred inputs,
    chained-dispatch slope (cancels per-call overhead)."""
    import time

    import jax
    from jax.experimental.shard_map import shard_map
    from jax.sharding import Mesh, NamedSharding, PartitionSpec

    from concourse import bass2jax, mybir as _mybir

    bass2jax.install_neuronx_cc_hook()
    nc = _get_program(reps)
    in_maps = _make_in_maps(**inputs)

    partition_name = nc.partition_id_tensor.name if nc.partition_id_tensor else None
    in_names, out_names, out_avals, zero_outs = [], [], [], []
    for alloc in nc.m.functions[0].allocations:
        if not isinstance(alloc, _mybir.MemoryLocationSet):
            continue
        name = alloc.memorylocations[0].name
        if alloc.kind == "ExternalInput":
            if name != partition_name:
                in_names.append(name)
        elif alloc.kind == "ExternalOutput":
            shape = tuple(alloc.tensor_shape)
            dtype = _mybir.dt.np(alloc.dtype)
            out_names.append(name)
            out_avals.append(jax.core.ShapedArray(shape, dtype))
            zero_outs.append(np.zeros(shape, dtype))
    n_params = len(in_names)
    all_in_names = list(in_names) + list(out_names)
    if partition_name is not None:
        all_in_names.append(partition_name)

    def _body(*args):
        operands = list(args)
        if partition_name is not None:
            operands.append(bass2jax.partition_id_tensor())
        outs = bass2jax._bass_exec_p.bind(
            *operands,
            out_avals=tuple(out_avals),
            in_names=tuple(all_in_names),
            out_names=tuple(out_names),
            lowering_input_output_aliases=(),
            sim_require_finite=True,
            sim_require_nnan=True,
            nc=nc,
        )
        return tuple(outs)

    n_outs = len(out_avals)
    donate = tuple(range(n_params, n_params + n_outs))
    devices = jax.devices()[:8]
    mesh = Mesh(np.asarray(devices), ("core",))
    in_specs = (PartitionSpec("core"),) * (n_params + n_outs)
    out_specs = (PartitionSpec("core"),) * n_outs
    sharded = jax.jit(
        shard_map(_body, mesh=mesh, in_specs=in_specs, out_specs=out_specs,
                  check_rep=False),
        donate_argnums=donate, keep_unused=True,
    )
    concat_in = [
        np.concatenate([np.asarray(in_maps[c][nm]) for c in range(8)], axis=0)
        for nm in in_names
    ]
    sh = NamedSharding(mesh, PartitionSpec("core"))
    dev_in = [jax.device_put(x, sh) for x in concat_in]

    def fresh_zeros():
        return [
            jax.device_put(np.zeros((8 * z.shape[0], *z.shape[1:]), z.dtype), sh)
            for z in zero_outs
        ]

    out = sharded(*dev_in, *fresh_zeros())
    jax.block_until_ready(out)

    def run_chain(k):
        zsets = [fresh_zeros() for _ in range(k)]
        for zs in zsets:
            jax.block_until_ready(zs)
        t0 = time.perf_counter()
        outs = [sharded(*dev_in, *zs) for zs in zsets]
        jax.block_until_ready(outs)
        return time.perf_counter() - t0

    run_chain(2)  # extra warmup
    slopes = []
    for _ in range(max(3, iters // 4)):
        t_a = run_chain(1)
        t_b = run_chain(chain)
        slopes.append((t_b - t_a) / (chain - 1.0))
    slopes.sort()
    med = slopes[len(slopes) // 2]
    print(f"bench(reps={reps}): slopes(us) = "
          f"{[f'{s*1e6:.0f}' for s in slopes]} -> median {med*1e6:.0f}us "
          f"min {slopes[0]*1e6:.0f}us")
    return med * 1e9


def bench_paired(inputs, pairs=10, hi_reps=8):
    """Paired-difference timing: interleave isolated calls of the reps=1 and
    reps=hi NEFFs; median of (t_hi - t_lo)/(hi-1) cancels slow drift."""
    import time

    r_lo = _BenchRunner(inputs, reps=1)
    r_hi = _BenchRunner(inputs, reps=hi_reps)
    r_lo.run(); r_hi.run(); r_lo.run(); r_hi.run()  # warm both
    diffs = []
    for _ in range(pairs):
        t_lo = r_lo.run()
        t_hi = r_hi.run()
        diffs.append((t_hi - t_lo) / (hi_reps - 1.0))
    diffs.sort()
    med = diffs[len(diffs) // 2]
    print(f"bench_paired: per-body diffs(us) = "
          f"{[f'{d*1e6:.0f}' for d in diffs]} -> median {med*1e6:.0f}us")
    return med * 1e9


class _BenchRunner:
    def __init__(self, inputs, reps):
        import jax
        from jax.experimental.shard_map import shard_map
        from jax.sharding import Mesh, NamedSharding, PartitionSpec
        from concourse import bass2jax, mybir as _mybir

        bass2jax.install_neuronx_cc_hook()
        nc = _get_program(reps)
        in_maps = _make_in_maps(**inputs)
        partition_name = (nc.partition_id_tensor.name
                          if nc.partition_id_tensor else None)
        in_names, out_names, out_avals, zero_outs = [], [], [], []
        for alloc in nc.m.functions[0].allocations:
            if not isinstance(alloc, _mybir.MemoryLocationSet):
                continue
            name = alloc.memorylocations[0].name
            if alloc.kind == "ExternalInput":
                if name != partition_name:
                    in_names.append(name)
            elif alloc.kind == "ExternalOutput":
                shape = tuple(alloc.tensor_shape)
                dtype = _mybir.dt.np(alloc.dtype)
                out_names.append(name)
                out_avals.append(jax.core.ShapedArray(shape, dtype))
                zero_outs.append(np.zeros(shape, dtype))
        n_params = len(in_names)
        all_in = list(in_names) + list(out_names)
        if partition_name is not None:
            all_in.append(partition_name)

        def _body(*args):
            operands = list(args)
            if partition_name is not None:
                operands.append(bass2jax.partition_id_tensor())
            return tuple(bass2jax._bass_exec_p.bind(
                *operands, out_avals=tuple(out_avals), in_names=tuple(all_in),
                out_names=tuple(out_names), lowering_input_output_aliases=(),
                sim_require_finite=True, sim_require_nnan=True, nc=nc))

        donate = tuple(range(n_params, n_params + len(out_avals)))
        devices = jax.devices()[:8]
        mesh = Mesh(np.asarray(devices), ("core",))
        specs = (PartitionSpec("core"),)
        self._sharded = jax.jit(
            shard_map(_body, mesh=mesh,
                      in_specs=specs * (n_params + len(out_avals)),
                      out_specs=specs * len(out_avals), check_rep=False),
            donate_argnums=donate, keep_unused=True)
        sh = NamedSharding(mesh, PartitionSpec("core"))
        self._dev_in = [jax.device_put(
            np.concatenate([np.asarray(in_maps[c][nm]) for c in range(8)],
                           axis=0), sh)
            for nm in in_names]
        self._zero_outs = zero_outs
        self._sh = sh
        self._jax = jax

    def run(self):
        import time
        jax = self._jax
        zs = [jax.device_put(
            np.zeros((8 * z.shape[0], *z.shape[1:]), z.dtype), self._sh)
            for z in self._zero_outs]
        jax.block_until_ready(zs)
        t0 = time.perf_counter()
        out = self._sharded(*self._dev_in, *zs)
        jax.block_until_ready(out)
        return time.perf_counter() - t0


# revision 24
# speedup vs baseline: 2.4308x; 1.2454x over previous
"""Multi-head cross-attention (post-LN) Trainium2 Bass kernel.

Full inputs -> full outputs. Sharding: 8 cores = 4 batches x 2 query-row
halves (512 rows each).  Host pre-transposes h/c/weights so every matmul
contraction dim lands on SBUF partitions with no on-chip transposes.

Per-core pipeline (all matmuls float32r: full PE rate at free-dim 512):
  ph1: qT[e,i]  = WqT.T @ hT            (e on partitions, stays in SBUF)
  ph2: v[j,e]   = cT.T @ WvT            -> DRAM scratch (plain layout)
  ph3 per head pair (cT resident):
       kT_hp[e128,j] = WkT_pair.T @ cT  (fused, no spill)
       sT[j,i] = kT.T @ qT ; pT = exp(SCALE*sT)  (ACT, fused scale)
       avT[d,i] + denom row = v_aug.T @ pT       (PSUM accum over j,
           v_aug = per-pair V tiles re-loaded with a ones column)
       avT normalized by 1/denom (PE outer-product broadcast), kept in SBUF
  ph4: attn_out[i,o] = avT.T @ WoT ; out = LN(attn_out + h) * gamma + beta
"""

import sys

for _p in ("/opt/trn_rl_repo", "/root/.axon_site/_ro/trn_rl_repo"):
    if _p not in sys.path:
        sys.path.append(_p)

import numpy as np

import concourse.bass as bass
import concourse.tile as tile
from concourse import bacc, mybir
from concourse.bass_utils import run_bass_kernel_spmd

P = 128
D = 1024          # d_model
I = 512           # query rows per core
J = 2048          # kv length
NH = 16           # heads
DH = 64           # head dim
SCALE = 1.0 / (DH ** 0.5)
LN_EPS = 1e-5
F32 = mybir.dt.float32
F32R = mybir.dt.float32r

MT = D // P       # 8 m-tiles (contraction over d_model)
ET = D // P       # 8 e-tiles (head features)
JT = J // P       # 16 j-tiles
JB = J // 512     # 4 j-blocks of 512
NPAIR = NH // 2   # 8 head pairs
NCHUNK = 8        # score chunks per head: 2 j-tiles each (2 PSUM banks)


def build_program(reps=1):
    nc = bacc.Bacc(None, target_bir_lowering=False, debug=False)

    hT = nc.dram_tensor("hT", [D, I], F32R, kind="ExternalInput")
    cT = nc.dram_tensor("cT", [D, J], F32R, kind="ExternalInput")
    wqT = nc.dram_tensor("wqT", [D, D], F32R, kind="ExternalInput")
    wkT = nc.dram_tensor("wkT", [D, D], F32R, kind="ExternalInput")
    wvT = nc.dram_tensor("wvT", [D, D], F32R, kind="ExternalInput")
    woT = nc.dram_tensor("woT", [D, D], F32R, kind="ExternalInput")
    hres = nc.dram_tensor("hres", [I, D], F32, kind="ExternalInput")
    gamma = nc.dram_tensor("gamma", [P, D], F32, kind="ExternalInput")
    beta = nc.dram_tensor("beta", [P, D], F32, kind="ExternalInput")
    out = nc.dram_tensor("out", [I, D], F32, kind="ExternalOutput")

    with tile.TileContext(nc) as tc:
        with (
            tc.tile_pool(name="consts", bufs=1) as consts,
            tc.tile_pool(name="persist", bufs=1) as persist,
            tc.tile_pool(name="psum", bufs=1, space="PSUM") as psum,
            tc.tile_pool(name="dram", bufs=1, space="DRAM") as dram,
        ):
            # ---- constants & persistents ----------------------------------
            gamma_bc = consts.tile([P, D], F32, tag="gamma_bc")
            beta_bc = consts.tile([P, D], F32, tag="beta_bc")
            nc.sync.dma_start(gamma_bc, gamma.ap())
            nc.sync.dma_start(beta_bc, beta.ap())
            ones_row = consts.tile([1, DH], F32R, tag="ones_row")
            nc.vector.memset(ones_row.bitcast(F32), 1.0)
            eps_t = consts.tile([P, 1], F32, tag="eps")
            nc.vector.memset(eps_t, LN_EPS)

            qT = persist.tile([P, ET, I], F32R, tag="qT")       # 16KB/part
            avT = persist.tile([P, ET, I], F32R, tag="avT")     # 16KB/part
            cT_t = persist.tile([P, MT, J], F32R, tag="cT")     # 64KB/part
            v_dram = dram.tile([JT, P, D], F32R)

            for _rep in range(reps):
                for mt in range(MT):  # split across DMA queues
                    nc.sync.dma_start(
                        cT_t[:, mt, :], cT.ap()[mt * P : (mt + 1) * P, :]
                    )

                # ===== ph1: Q projection ===============================
                with tc.tile_pool(name="ph1", bufs=1) as ph1pool:
                    hT_t = ph1pool.tile([P, MT, I], F32R, tag="hT")
                    for mt in range(MT):
                        nc.sync.dma_start(
                            hT_t[:, mt, :], hT.ap()[mt * P : (mt + 1) * P, :]
                        )
                    wq_t = []
                    for mt in range(MT):
                        w = ph1pool.tile([P, D], F32R, tag=f"wq{mt}")
                        nc.sync.dma_start(w, wqT.ap()[mt * P : (mt + 1) * P, :])
                        wq_t.append(w)
                    for et in range(ET):
                        ps = psum.tile([P, 512], F32, tag=("vps", "kps")[et % 2],
                                       name="q_ps")
                        for mt in range(MT):
                            nc.tensor.matmul(
                                ps,
                                wq_t[mt][:, et * P : (et + 1) * P],
                                hT_t[:, mt, :],
                                start=(mt == 0),
                                stop=(mt == MT - 1),
                            )
                        nc.vector.tensor_copy(qT[:, et, :], ps)

                # ===== ph2: V projection -> DRAM =======================
                with tc.tile_pool(name="ph23", bufs=2) as ph3pool:
                    ph2pool = ph3pool
                    for eh in range(2):
                        wv_t = []
                        for mt in range(MT):
                            w = ph2pool.tile([P, D // 2], F32R, tag=f"wv{mt}",
                                             name="wv", bufs=1)
                            nc.sync.dma_start(
                                w, wvT.ap()[mt * P : (mt + 1) * P,
                                            eh * (D // 2) : (eh + 1) * (D // 2)]
                            )
                            wv_t.append(w)
                        for jt in range(JT):
                            ps = psum.tile([P, 512], F32,
                                           tag="vps", name="v_ps")
                            for mt in range(MT):
                                nc.tensor.matmul(
                                    ps,
                                    cT_t[:, mt, jt * P : (jt + 1) * P],
                                    wv_t[mt],
                                    start=(mt == 0),
                                    stop=(mt == MT - 1),
                                )
                            vs = ph2pool.tile([P, 512], F32R, tag="vstage",
                                              name="vstage", bufs=2)
                            nc.vector.tensor_copy(vs, ps)
                            nc.sync.dma_start(
                                v_dram[jt, :, eh * 512 : (eh + 1) * 512], vs
                            )

                    # ===== ph3: attention per head pair ================
                    for hp in range(NPAIR):
                        # K^T for this pair, fused (no spill)
                        wk_t = []
                        for mt in range(MT):
                            w = ph3pool.tile([P, P], F32R, tag=f"wk{mt}",
                                             name="wk", bufs=2)
                            nc.sync.dma_start(
                                w, wkT.ap()[mt * P : (mt + 1) * P,
                                            hp * P : (hp + 1) * P]
                            )
                            wk_t.append(w)
                        kT_hp = ph3pool.tile([P, J], F32R, tag="kT_hp")
                        for jb in range(JB):
                            kps = psum.tile([P, 512], F32, tag="kps", name="kps")
                            for mt in range(MT):
                                nc.tensor.matmul(
                                    kps,
                                    wk_t[mt],
                                    cT_t[:, mt, jb * 512 : (jb + 1) * 512],
                                    start=(mt == 0),
                                    stop=(mt == MT - 1),
                                )
                            nc.vector.tensor_copy(
                                kT_hp[:, jb * 512 : (jb + 1) * 512], kps
                            )

                        # V tiles for this pair, ones-augmented
                        v_hp = ph3pool.tile([P, JT, 2, DH + 1], F32R, tag="v_hp")
                        nc.vector.memset(
                            v_hp[:, :, :, DH : DH + 1].bitcast(F32), 1.0
                        )
                        for h in range(2):
                            nc.sync.dma_start(
                                v_hp[:, :, h, 0:DH],
                                v_dram[:, :, (2 * hp + h) * DH :
                                       (2 * hp + h + 1) * DH]
                                .rearrange("jt p d -> p jt d"),
                            )

                        q_pair = qT[:, hp, :]
                        bounds = ((0, DH), (DH, P))
                        av_ps = [
                            psum.tile([P, I], F32, tag=("avA", "avB")[hi],
                                      name="avp")
                            for hi in range(2)
                        ]
                        for ci in range(NCHUNK):
                            scs = [
                                psum.tile([P, 2, 512], F32,
                                          tag=("scA", "scB")[hi], name="sc")
                                for hi in range(2)
                            ]
                            # interleave A/B so the K=64 matmuls row-pack on
                            # disjoint halves of the PE array
                            for k in range(2):
                                jt = 2 * ci + k
                                for hi, (p0, p1) in enumerate(bounds):
                                    nc.tensor.matmul(
                                        scs[hi][:, k, :],
                                        kT_hp[p0:p1, jt * P : (jt + 1) * P],
                                        q_pair[p0:p1, :],
                                        start=True,
                                        stop=True,
                                    )
                            pTs = []
                            for hi in range(2):
                                pT = ph3pool.tile([P, 2, 512], F32R,
                                                  tag=("pA", "pB")[hi], name="pT")
                                nc.scalar.activation(
                                    pT.rearrange("p a b -> p (a b)"),
                                    scs[hi].rearrange("p a b -> p (a b)"),
                                    mybir.ActivationFunctionType.Exp,
                                    scale=SCALE,
                                )
                                pTs.append(pT)
                            for k in range(2):
                                jt = 2 * ci + k
                                for hi in range(2):
                                    nc.tensor.matmul(
                                        av_ps[hi][0 : DH + 1, :],
                                        v_hp[:, jt, hi, :],
                                        pTs[hi][:, k, :],
                                        start=(jt == 0),
                                        stop=(jt == JT - 1),
                                    )

                        for hi in range(2):
                            recip = ph3pool.tile([1, I], F32R, tag="recip",
                                                 name="recip")
                            with nc.allow_low_precision(
                                reason="f32r keeps the f32 mantissa in SBUF"
                            ):
                                nc.vector.reciprocal(
                                    recip, av_ps[hi][DH : DH + 1, :]
                                )
                            # replicate [1, I] across DH partitions on the PE
                            rbc_ps = psum.tile([DH, I], F32,
                                               tag=("scA", "scB")[hi],
                                               name="rbc_ps")
                            nc.tensor.matmul(rbc_ps, ones_row, recip,
                                             start=True, stop=True)
                            rbc = ph3pool.tile([DH, I], F32, tag="rbc",
                                               name="rbc")
                            nc.vector.tensor_copy(rbc, rbc_ps)
                            nc.vector.tensor_tensor(
                                avT[hi * DH : (hi + 1) * DH, hp, :],
                                av_ps[hi][0:DH, :],
                                rbc,
                                mybir.AluOpType.mult,
                            )

                # ===== ph4: out-proj + residual + LN ===================
                with (
                    tc.tile_pool(name="ph4w", bufs=1) as ph4w,
                    tc.tile_pool(name="ph4", bufs=2) as ph4pool,
                ):
                    wo_t = []
                    for et in range(ET):
                        w = ph4w.tile([P, D], F32R, tag=f"wo{et}")
                        nc.sync.dma_start(w, woT.ap()[et * P : (et + 1) * P, :])
                        wo_t.append(w)
                    for it in range(I // P):
                        po = psum.tile([P, 2, 512], F32,
                                       tag=("scA", "scB")[it % 2], name="po")
                        for ob in range(2):
                            for et in range(ET):
                                nc.tensor.matmul(
                                    po[:, ob, :],
                                    avT[:, et, it * P : (it + 1) * P],
                                    wo_t[et][:, ob * 512 : (ob + 1) * 512],
                                    start=(et == 0),
                                    stop=(et == ET - 1),
                                )
                        hres_t = ph4pool.tile([P, D], F32, tag="hres")
                        nc.sync.dma_start(hres_t,
                                          hres.ap()[it * P : (it + 1) * P, :])
                        x = ph4pool.tile([P, D], F32, tag="x")
                        nc.vector.tensor_tensor(
                            x, po.rearrange("p a b -> p (a b)"), hres_t,
                            mybir.AluOpType.add,
                        )
                        stats = ph4pool.tile([P, 2, nc.vector.BN_STATS_DIM], F32,
                                             tag="stats")
                        xg = x.rearrange("p (g d) -> p g d", g=2)
                        for g in range(2):
                            nc.vector.bn_stats(stats[:, g, :], xg[:, g, :])
                        mv = ph4pool.tile([P, nc.vector.BN_AGGR_DIM], F32,
                                          tag="mv")
                        nc.vector.bn_aggr(mv, stats)
                        rstd = ph4pool.tile([P, 1], F32, tag="rstd")
                        nc.scalar.activation(
                            rstd, mv[:, 1:2], mybir.ActivationFunctionType.Sqrt,
                            bias=eps_t,
                        )
                        nc.vector.reciprocal(rstd, rstd)
                        nc.vector.tensor_scalar(
                            x, x, mv[:, 0:1], rstd,
                            op0=mybir.AluOpType.subtract,
                            op1=mybir.AluOpType.mult,
                        )
                        nc.vector.tensor_tensor(x, x, gamma_bc,
                                                mybir.AluOpType.mult)
                        nc.vector.tensor_tensor(x, x, beta_bc,
                                                mybir.AluOpType.add)
                        nc.sync.dma_start(out.ap()[it * P : (it + 1) * P, :], x)

    nc.compile()
    return nc


_NC_CACHE = {}


def _get_program(reps=1):
    if reps not in _NC_CACHE:
        _NC_CACHE[reps] = build_program(reps)
    return _NC_CACHE[reps]


def _make_in_maps(h, c, Wq, Wkv, Wo, gamma, beta):
    h = np.asarray(h, dtype=np.float32)
    c = np.asarray(c, dtype=np.float32)
    Wq = np.asarray(Wq, dtype=np.float32)
    Wkv = np.asarray(Wkv, dtype=np.float32)
    Wo = np.asarray(Wo, dtype=np.float32)
    gamma = np.asarray(gamma, dtype=np.float32)
    beta = np.asarray(beta, dtype=np.float32)

    q_len, batch, d_model = h.shape
    assert (q_len, batch, d_model) == (1024, 4, D)

    wqT = np.ascontiguousarray(Wq.T)
    wkT = np.ascontiguousarray(Wkv[:D].T)
    wvT = np.ascontiguousarray(Wkv[D:].T)
    woT = np.ascontiguousarray(Wo.T)
    gamma_b = np.ascontiguousarray(np.broadcast_to(gamma, (P, D)))
    beta_b = np.ascontiguousarray(np.broadcast_to(beta, (P, D)))

    in_maps = []
    for core in range(8):
        b, g = divmod(core, 2)
        i0, i1 = g * I, (g + 1) * I
        in_maps.append({
            "hT": np.ascontiguousarray(h[i0:i1, b, :].T),
            "cT": np.ascontiguousarray(c[:, b, :].T),
            "wqT": wqT,
            "wkT": wkT,
            "wvT": wvT,
            "woT": woT,
            "hres": np.ascontiguousarray(h[i0:i1, b, :]),
            "gamma": gamma_b,
            "beta": beta_b,
        })
    return in_maps


def kernel(h, c, Wq, Wkv, Wo, gamma, beta):
    in_maps = _make_in_maps(h, c, Wq, Wkv, Wo, gamma, beta)
    nc = _get_program()
    res = run_bass_kernel_spmd(nc, in_maps, core_ids=list(range(8)))

    q_len, batch = 1024, 4
    out = np.empty((q_len, batch, D), dtype=np.float32)
    for core in range(8):
        b, g = divmod(core, 2)
        out[g * I : (g + 1) * I, b, :] = res.results[core]["out"]
    return out


def bench(inputs, iters=20, reps=1, chain=8):
    """Time the on-device execution: warm jit + pre-transferred inputs,
    chained-dispatch slope (cancels per-call overhead)."""
    import time

    import jax
    from jax.experimental.shard_map import shard_map
    from jax.sharding import Mesh, NamedSharding, PartitionSpec

    from concourse import bass2jax, mybir as _mybir

    bass2jax.install_neuronx_cc_hook()
    nc = _get_program(reps)
    in_maps = _make_in_maps(**inputs)

    partition_name = nc.partition_id_tensor.name if nc.partition_id_tensor else None
    in_names, out_names, out_avals, zero_outs = [], [], [], []
    for alloc in nc.m.functions[0].allocations:
        if not isinstance(alloc, _mybir.MemoryLocationSet):
            continue
        name = alloc.memorylocations[0].name
        if alloc.kind == "ExternalInput":
            if name != partition_name:
                in_names.append(name)
        elif alloc.kind == "ExternalOutput":
            shape = tuple(alloc.tensor_shape)
            dtype = _mybir.dt.np(alloc.dtype)
            out_names.append(name)
            out_avals.append(jax.core.ShapedArray(shape, dtype))
            zero_outs.append(np.zeros(shape, dtype))
    n_params = len(in_names)
    all_in_names = list(in_names) + list(out_names)
    if partition_name is not None:
        all_in_names.append(partition_name)

    def _body(*args):
        operands = list(args)
        if partition_name is not None:
            operands.append(bass2jax.partition_id_tensor())
        outs = bass2jax._bass_exec_p.bind(
            *operands,
            out_avals=tuple(out_avals),
            in_names=tuple(all_in_names),
            out_names=tuple(out_names),
            lowering_input_output_aliases=(),
            sim_require_finite=True,
            sim_require_nnan=True,
            nc=nc,
        )
        return tuple(outs)

    n_outs = len(out_avals)
    donate = tuple(range(n_params, n_params + n_outs))
    devices = jax.devices()[:8]
    mesh = Mesh(np.asarray(devices), ("core",))
    in_specs = (PartitionSpec("core"),) * (n_params + n_outs)
    out_specs = (PartitionSpec("core"),) * n_outs
    sharded = jax.jit(
        shard_map(_body, mesh=mesh, in_specs=in_specs, out_specs=out_specs,
                  check_rep=False),
        donate_argnums=donate, keep_unused=True,
    )
    concat_in = [
        np.concatenate([np.asarray(in_maps[c][nm]) for c in range(8)], axis=0)
        for nm in in_names
    ]
    sh = NamedSharding(mesh, PartitionSpec("core"))
    dev_in = [jax.device_put(x, sh) for x in concat_in]

    def fresh_zeros():
        return [
            jax.device_put(np.zeros((8 * z.shape[0], *z.shape[1:]), z.dtype), sh)
            for z in zero_outs
        ]

    out = sharded(*dev_in, *fresh_zeros())
    jax.block_until_ready(out)

    def run_chain(k):
        zsets = [fresh_zeros() for _ in range(k)]
        for zs in zsets:
            jax.block_until_ready(zs)
        t0 = time.perf_counter()
        outs = [sharded(*dev_in, *zs) for zs in zsets]
        jax.block_until_ready(outs)
        return time.perf_counter() - t0

    run_chain(2)  # extra warmup
    slopes = []
    for _ in range(max(3, iters // 4)):
        t_a = run_chain(1)
        t_b = run_chain(chain)
        slopes.append((t_b - t_a) / (chain - 1.0))
    slopes.sort()
    med = slopes[len(slopes) // 2]
    print(f"bench(reps={reps}): slopes(us) = "
          f"{[f'{s*1e6:.0f}' for s in slopes]} -> median {med*1e6:.0f}us "
          f"min {slopes[0]*1e6:.0f}us")
    return med * 1e9


def bench_paired(inputs, pairs=10, hi_reps=8):
    """Paired-difference timing: interleave isolated calls of the reps=1 and
    reps=hi NEFFs; median of (t_hi - t_lo)/(hi-1) cancels slow drift."""
    import time

    r_lo = _BenchRunner(inputs, reps=1)
    r_hi = _BenchRunner(inputs, reps=hi_reps)
    r_lo.run(); r_hi.run(); r_lo.run(); r_hi.run()  # warm both
    diffs = []
    for _ in range(pairs):
        t_lo = r_lo.run()
        t_hi = r_hi.run()
        diffs.append((t_hi - t_lo) / (hi_reps - 1.0))
    diffs.sort()
    med = diffs[len(diffs) // 2]
    print(f"bench_paired: per-body diffs(us) = "
          f"{[f'{d*1e6:.0f}' for d in diffs]} -> median {med*1e6:.0f}us")
    return med * 1e9


class _BenchRunner:
    def __init__(self, inputs, reps):
        import jax
        from jax.experimental.shard_map import shard_map
        from jax.sharding import Mesh, NamedSharding, PartitionSpec
        from concourse import bass2jax, mybir as _mybir

        bass2jax.install_neuronx_cc_hook()
        nc = _get_program(reps)
        in_maps = _make_in_maps(**inputs)
        partition_name = (nc.partition_id_tensor.name
                          if nc.partition_id_tensor else None)
        in_names, out_names, out_avals, zero_outs = [], [], [], []
        for alloc in nc.m.functions[0].allocations:
            if not isinstance(alloc, _mybir.MemoryLocationSet):
                continue
            name = alloc.memorylocations[0].name
            if alloc.kind == "ExternalInput":
                if name != partition_name:
                    in_names.append(name)
            elif alloc.kind == "ExternalOutput":
                shape = tuple(alloc.tensor_shape)
                dtype = _mybir.dt.np(alloc.dtype)
                out_names.append(name)
                out_avals.append(jax.core.ShapedArray(shape, dtype))
                zero_outs.append(np.zeros(shape, dtype))
        n_params = len(in_names)
        all_in = list(in_names) + list(out_names)
        if partition_name is not None:
            all_in.append(partition_name)

        def _body(*args):
            operands = list(args)
            if partition_name is not None:
                operands.append(bass2jax.partition_id_tensor())
            return tuple(bass2jax._bass_exec_p.bind(
                *operands, out_avals=tuple(out_avals), in_names=tuple(all_in),
                out_names=tuple(out_names), lowering_input_output_aliases=(),
                sim_require_finite=True, sim_require_nnan=True, nc=nc))

        donate = tuple(range(n_params, n_params + len(out_avals)))
        devices = jax.devices()[:8]
        mesh = Mesh(np.asarray(devices), ("core",))
        specs = (PartitionSpec("core"),)
        self._sharded = jax.jit(
            shard_map(_body, mesh=mesh,
                      in_specs=specs * (n_params + len(out_avals)),
                      out_specs=specs * len(out_avals), check_rep=False),
            donate_argnums=donate, keep_unused=True)
        sh = NamedSharding(mesh, PartitionSpec("core"))
        self._dev_in = [jax.device_put(
            np.concatenate([np.asarray(in_maps[c][nm]) for c in range(8)],
                           axis=0), sh)
            for nm in in_names]
        self._zero_outs = zero_outs
        self._sh = sh
        self._jax = jax

    def run(self):
        import time
        jax = self._jax
        zs = [jax.device_put(
            np.zeros((8 * z.shape[0], *z.shape[1:]), z.dtype), self._sh)
            for z in self._zero_outs]
        jax.block_until_ready(zs)
        t0 = time.perf_counter()
        out = self._sharded(*self._dev_in, *zs)
        jax.block_until_ready(out)
        return time.perf_counter() - t0


# revision 25
# speedup vs baseline: 2.5514x; 1.0496x over previous
"""Multi-head cross-attention (post-LN) Trainium2 Bass kernel.

Full inputs -> full outputs. Sharding: 8 cores = 4 batches x 2 query-row
halves (512 rows each).  Host pre-transposes h/c/weights so every matmul
contraction dim lands on SBUF partitions with no on-chip transposes.

Per-core pipeline (all matmuls float32r: full PE rate at free-dim 512):
  ph1: qT[e,i]  = WqT.T @ hT            (e on partitions, stays in SBUF)
  ph2: v[j,e]   = cT.T @ WvT            -> DRAM scratch (plain layout)
  ph3 per head pair (cT resident):
       kT_hp[e128,j] = WkT_pair.T @ cT  (fused, no spill)
       sT[j,i] = kT.T @ qT ; pT = exp(SCALE*sT)  (ACT, fused scale)
       avT[d,i] + denom row = v_aug.T @ pT       (PSUM accum over j,
           v_aug = per-pair V tiles re-loaded with a ones column)
       avT normalized by 1/denom (PE outer-product broadcast), kept in SBUF
  ph4: attn_out[i,o] = avT.T @ WoT ; out = LN(attn_out + h) * gamma + beta
"""

import sys

for _p in ("/opt/trn_rl_repo", "/root/.axon_site/_ro/trn_rl_repo"):
    if _p not in sys.path:
        sys.path.append(_p)

import numpy as np

import concourse.bass as bass
import concourse.tile as tile
from concourse import bacc, mybir
from concourse.bass_utils import run_bass_kernel_spmd

P = 128
D = 1024          # d_model
I = 512           # query rows per core
J = 2048          # kv length
NH = 16           # heads
DH = 64           # head dim
SCALE = 1.0 / (DH ** 0.5)
LN_EPS = 1e-5
F32 = mybir.dt.float32
F32R = mybir.dt.float32r

MT = D // P       # 8 m-tiles (contraction over d_model)
ET = D // P       # 8 e-tiles (head features)
JT = J // P       # 16 j-tiles
JB = J // 512     # 4 j-blocks of 512
NPAIR = NH // 2   # 8 head pairs
NCHUNK = 8        # score chunks per head: 2 j-tiles each (2 PSUM banks)


def build_program(reps=1):
    nc = bacc.Bacc(None, target_bir_lowering=False, debug=False)

    hT = nc.dram_tensor("hT", [D, I], F32R, kind="ExternalInput")
    cT = nc.dram_tensor("cT", [D, J], F32R, kind="ExternalInput")
    wqT = nc.dram_tensor("wqT", [D, D], F32R, kind="ExternalInput")
    wkT = nc.dram_tensor("wkT", [D, D], F32R, kind="ExternalInput")
    wvT = nc.dram_tensor("wvT", [D, D], F32R, kind="ExternalInput")
    woT = nc.dram_tensor("woT", [D, D], F32R, kind="ExternalInput")
    hres = nc.dram_tensor("hres", [I, D], F32, kind="ExternalInput")
    gamma = nc.dram_tensor("gamma", [P, D], F32, kind="ExternalInput")
    beta = nc.dram_tensor("beta", [P, D], F32, kind="ExternalInput")
    out = nc.dram_tensor("out", [I, D], F32, kind="ExternalOutput")

    with tile.TileContext(nc) as tc:
        with (
            tc.tile_pool(name="consts", bufs=1) as consts,
            tc.tile_pool(name="persist", bufs=1) as persist,
            tc.tile_pool(name="psum", bufs=1, space="PSUM") as psum,
            tc.tile_pool(name="dram", bufs=1, space="DRAM") as dram,
        ):
            # ---- constants & persistents ----------------------------------
            gamma_bc = consts.tile([P, D], F32, tag="gamma_bc")
            beta_bc = consts.tile([P, D], F32, tag="beta_bc")
            ones_row = consts.tile([1, DH], F32R, tag="ones_row")
            nc.vector.memset(ones_row.bitcast(F32), 1.0)
            eps_t = consts.tile([P, 1], F32, tag="eps")
            nc.vector.memset(eps_t, LN_EPS)

            qT = persist.tile([P, ET, I], F32R, tag="qT")       # 16KB/part
            avT = persist.tile([P, ET, I], F32R, tag="avT")     # 16KB/part
            cT_t = persist.tile([P, MT, J], F32R, tag="cT")     # 64KB/part
            v_dram = dram.tile([JT, P, D], F32R)

            for _rep in range(reps):
                # ===== ph1: Q projection ===============================
                with tc.tile_pool(name="ph1", bufs=1) as ph1pool:
                    # ph1's operands first so the PE starts ASAP; the bigger
                    # cT load streams behind them and lands before ph2.
                    hT_t = ph1pool.tile([P, MT, I], F32R, tag="hT")
                    for mt in range(MT):
                        nc.sync.dma_start(
                            hT_t[:, mt, :], hT.ap()[mt * P : (mt + 1) * P, :]
                        )
                    wq_t = []
                    for mt in range(MT):
                        w = ph1pool.tile([P, D], F32R, tag=f"wq{mt}")
                        nc.sync.dma_start(w, wqT.ap()[mt * P : (mt + 1) * P, :])
                        wq_t.append(w)
                    for mt in range(MT):  # split across DMA queues
                        nc.sync.dma_start(
                            cT_t[:, mt, :], cT.ap()[mt * P : (mt + 1) * P, :]
                        )
                    nc.sync.dma_start(gamma_bc, gamma.ap())
                    nc.sync.dma_start(beta_bc, beta.ap())
                    for et in range(ET):
                        ps = psum.tile([P, 512], F32, tag=("vps", "kps")[et % 2],
                                       name="q_ps")
                        for mt in range(MT):
                            nc.tensor.matmul(
                                ps,
                                wq_t[mt][:, et * P : (et + 1) * P],
                                hT_t[:, mt, :],
                                start=(mt == 0),
                                stop=(mt == MT - 1),
                            )
                        nc.vector.tensor_copy(qT[:, et, :], ps)

                # ===== ph2: V projection -> DRAM =======================
                with tc.tile_pool(name="ph23", bufs=2) as ph3pool:
                    ph2pool = ph3pool
                    for eh in range(2):
                        wv_t = []
                        for mt in range(MT):
                            w = ph2pool.tile([P, D // 2], F32R, tag=f"wv{mt}",
                                             name="wv", bufs=1)
                            nc.sync.dma_start(
                                w, wvT.ap()[mt * P : (mt + 1) * P,
                                            eh * (D // 2) : (eh + 1) * (D // 2)]
                            )
                            wv_t.append(w)
                        for jt in range(JT):
                            ps = psum.tile([P, 512], F32,
                                           tag="vps", name="v_ps")
                            for mt in range(MT):
                                nc.tensor.matmul(
                                    ps,
                                    cT_t[:, mt, jt * P : (jt + 1) * P],
                                    wv_t[mt],
                                    start=(mt == 0),
                                    stop=(mt == MT - 1),
                                )
                            vs = ph2pool.tile([P, 512], F32R, tag="vstage",
                                              name="vstage", bufs=2)
                            nc.vector.tensor_copy(vs, ps)
                            nc.sync.dma_start(
                                v_dram[jt, :, eh * 512 : (eh + 1) * 512], vs
                            )

                    # ===== ph3: attention per head pair ================
                    for hp in range(NPAIR):
                        # K^T for this pair, fused (no spill)
                        wk_t = []
                        for mt in range(MT):
                            w = ph3pool.tile([P, P], F32R, tag=f"wk{mt}",
                                             name="wk", bufs=2)
                            nc.sync.dma_start(
                                w, wkT.ap()[mt * P : (mt + 1) * P,
                                            hp * P : (hp + 1) * P]
                            )
                            wk_t.append(w)
                        kT_hp = ph3pool.tile([P, J], F32R, tag="kT_hp")
                        for jb in range(JB):
                            kps = psum.tile([P, 512], F32, tag="kps", name="kps")
                            for mt in range(MT):
                                nc.tensor.matmul(
                                    kps,
                                    wk_t[mt],
                                    cT_t[:, mt, jb * 512 : (jb + 1) * 512],
                                    start=(mt == 0),
                                    stop=(mt == MT - 1),
                                )
                            nc.vector.tensor_copy(
                                kT_hp[:, jb * 512 : (jb + 1) * 512], kps
                            )

                        # V tiles for this pair, ones-augmented
                        v_hp = ph3pool.tile([P, JT, 2, DH + 1], F32R, tag="v_hp")
                        nc.vector.memset(
                            v_hp[:, :, :, DH : DH + 1].bitcast(F32), 1.0
                        )
                        for h in range(2):
                            nc.sync.dma_start(
                                v_hp[:, :, h, 0:DH],
                                v_dram[:, :, (2 * hp + h) * DH :
                                       (2 * hp + h + 1) * DH]
                                .rearrange("jt p d -> p jt d"),
                            )

                        q_pair = qT[:, hp, :]
                        bounds = ((0, DH), (DH, P))
                        av_ps = [
                            psum.tile([P, I], F32, tag=("avA", "avB")[hi],
                                      name="avp")
                            for hi in range(2)
                        ]
                        for ci in range(NCHUNK):
                            scs = [
                                psum.tile([P, 2, 512], F32,
                                          tag=("scA", "scB")[hi], name="sc")
                                for hi in range(2)
                            ]
                            # interleave A/B so the K=64 matmuls row-pack on
                            # disjoint halves of the PE array
                            for k in range(2):
                                jt = 2 * ci + k
                                for hi, (p0, p1) in enumerate(bounds):
                                    nc.tensor.matmul(
                                        scs[hi][:, k, :],
                                        kT_hp[p0:p1, jt * P : (jt + 1) * P],
                                        q_pair[p0:p1, :],
                                        start=True,
                                        stop=True,
                                    )
                            pTs = []
                            for hi in range(2):
                                pT = ph3pool.tile([P, 2, 512], F32R,
                                                  tag=("pA", "pB")[hi], name="pT")
                                nc.scalar.activation(
                                    pT.rearrange("p a b -> p (a b)"),
                                    scs[hi].rearrange("p a b -> p (a b)"),
                                    mybir.ActivationFunctionType.Exp,
                                    scale=SCALE,
                                )
                                pTs.append(pT)
                            for k in range(2):
                                jt = 2 * ci + k
                                for hi in range(2):
                                    nc.tensor.matmul(
                                        av_ps[hi][0 : DH + 1, :],
                                        v_hp[:, jt, hi, :],
                                        pTs[hi][:, k, :],
                                        start=(jt == 0),
                                        stop=(jt == JT - 1),
                                    )

                        for hi in range(2):
                            recip = ph3pool.tile([1, I], F32R, tag="recip",
                                                 name="recip")
                            with nc.allow_low_precision(
                                reason="f32r keeps the f32 mantissa in SBUF"
                            ):
                                nc.vector.reciprocal(
                                    recip, av_ps[hi][DH : DH + 1, :]
                                )
                            # replicate [1, I] across DH partitions on the PE
                            rbc_ps = psum.tile([DH, I], F32,
                                               tag=("scA", "scB")[hi],
                                               name="rbc_ps")
                            nc.tensor.matmul(rbc_ps, ones_row, recip,
                                             start=True, stop=True)
                            rbc = ph3pool.tile([DH, I], F32, tag="rbc",
                                               name="rbc")
                            nc.vector.tensor_copy(rbc, rbc_ps)
                            nc.vector.tensor_tensor(
                                avT[hi * DH : (hi + 1) * DH, hp, :],
                                av_ps[hi][0:DH, :],
                                rbc,
                                mybir.AluOpType.mult,
                            )

                # ===== ph4: out-proj + residual + LN ===================
                with (
                    tc.tile_pool(name="ph4w", bufs=1) as ph4w,
                    tc.tile_pool(name="ph4", bufs=2) as ph4pool,
                ):
                    wo_t = []
                    for et in range(ET):
                        w = ph4w.tile([P, D], F32R, tag=f"wo{et}")
                        nc.sync.dma_start(w, woT.ap()[et * P : (et + 1) * P, :])
                        wo_t.append(w)
                    for it in range(I // P):
                        po = psum.tile([P, 2, 512], F32,
                                       tag=("scA", "scB")[it % 2], name="po")
                        for ob in range(2):
                            for et in range(ET):
                                nc.tensor.matmul(
                                    po[:, ob, :],
                                    avT[:, et, it * P : (it + 1) * P],
                                    wo_t[et][:, ob * 512 : (ob + 1) * 512],
                                    start=(et == 0),
                                    stop=(et == ET - 1),
                                )
                        hres_t = ph4pool.tile([P, D], F32, tag="hres")
                        nc.sync.dma_start(hres_t,
                                          hres.ap()[it * P : (it + 1) * P, :])
                        x = ph4pool.tile([P, D], F32, tag="x")
                        nc.vector.tensor_tensor(
                            x, po.rearrange("p a b -> p (a b)"), hres_t,
                            mybir.AluOpType.add,
                        )
                        stats = ph4pool.tile([P, 2, nc.vector.BN_STATS_DIM], F32,
                                             tag="stats")
                        xg = x.rearrange("p (g d) -> p g d", g=2)
                        for g in range(2):
                            nc.vector.bn_stats(stats[:, g, :], xg[:, g, :])
                        mv = ph4pool.tile([P, nc.vector.BN_AGGR_DIM], F32,
                                          tag="mv")
                        nc.vector.bn_aggr(mv, stats)
                        rstd = ph4pool.tile([P, 1], F32, tag="rstd")
                        nc.scalar.activation(
                            rstd, mv[:, 1:2], mybir.ActivationFunctionType.Sqrt,
                            bias=eps_t,
                        )
                        nc.vector.reciprocal(rstd, rstd)
                        nc.vector.tensor_scalar(
                            x, x, mv[:, 0:1], rstd,
                            op0=mybir.AluOpType.subtract,
                            op1=mybir.AluOpType.mult,
                        )
                        nc.vector.tensor_tensor(x, x, gamma_bc,
                                                mybir.AluOpType.mult)
                        nc.vector.tensor_tensor(x, x, beta_bc,
                                                mybir.AluOpType.add)
                        nc.sync.dma_start(out.ap()[it * P : (it + 1) * P, :], x)

    nc.compile()
    return nc


_NC_CACHE = {}


def _get_program(reps=1):
    if reps not in _NC_CACHE:
        _NC_CACHE[reps] = build_program(reps)
    return _NC_CACHE[reps]


def _make_in_maps(h, c, Wq, Wkv, Wo, gamma, beta):
    h = np.asarray(h, dtype=np.float32)
    c = np.asarray(c, dtype=np.float32)
    Wq = np.asarray(Wq, dtype=np.float32)
    Wkv = np.asarray(Wkv, dtype=np.float32)
    Wo = np.asarray(Wo, dtype=np.float32)
    gamma = np.asarray(gamma, dtype=np.float32)
    beta = np.asarray(beta, dtype=np.float32)

    q_len, batch, d_model = h.shape
    assert (q_len, batch, d_model) == (1024, 4, D)

    wqT = np.ascontiguousarray(Wq.T)
    wkT = np.ascontiguousarray(Wkv[:D].T)
    wvT = np.ascontiguousarray(Wkv[D:].T)
    woT = np.ascontiguousarray(Wo.T)
    gamma_b = np.ascontiguousarray(np.broadcast_to(gamma, (P, D)))
    beta_b = np.ascontiguousarray(np.broadcast_to(beta, (P, D)))

    in_maps = []
    for core in range(8):
        b, g = divmod(core, 2)
        i0, i1 = g * I, (g + 1) * I
        in_maps.append({
            "hT": np.ascontiguousarray(h[i0:i1, b, :].T),
            "cT": np.ascontiguousarray(c[:, b, :].T),
            "wqT": wqT,
            "wkT": wkT,
            "wvT": wvT,
            "woT": woT,
            "hres": np.ascontiguousarray(h[i0:i1, b, :]),
            "gamma": gamma_b,
            "beta": beta_b,
        })
    return in_maps


def kernel(h, c, Wq, Wkv, Wo, gamma, beta):
    in_maps = _make_in_maps(h, c, Wq, Wkv, Wo, gamma, beta)
    nc = _get_program()
    res = run_bass_kernel_spmd(nc, in_maps, core_ids=list(range(8)))

    q_len, batch = 1024, 4
    out = np.empty((q_len, batch, D), dtype=np.float32)
    for core in range(8):
        b, g = divmod(core, 2)
        out[g * I : (g + 1) * I, b, :] = res.results[core]["out"]
    return out


def bench(inputs, iters=20, reps=1, chain=8):
    """Time the on-device execution: warm jit + pre-transferred inputs,
    chained-dispatch slope (cancels per-call overhead)."""
    import time

    import jax
    from jax.experimental.shard_map import shard_map
    from jax.sharding import Mesh, NamedSharding, PartitionSpec

    from concourse import bass2jax, mybir as _mybir

    bass2jax.install_neuronx_cc_hook()
    nc = _get_program(reps)
    in_maps = _make_in_maps(**inputs)

    partition_name = nc.partition_id_tensor.name if nc.partition_id_tensor else None
    in_names, out_names, out_avals, zero_outs = [], [], [], []
    for alloc in nc.m.functions[0].allocations:
        if not isinstance(alloc, _mybir.MemoryLocationSet):
            continue
        name = alloc.memorylocations[0].name
        if alloc.kind == "ExternalInput":
            if name != partition_name:
                in_names.append(name)
        elif alloc.kind == "ExternalOutput":
            shape = tuple(alloc.tensor_shape)
            dtype = _mybir.dt.np(alloc.dtype)
            out_names.append(name)
            out_avals.append(jax.core.ShapedArray(shape, dtype))
            zero_outs.append(np.zeros(shape, dtype))
    n_params = len(in_names)
    all_in_names = list(in_names) + list(out_names)
    if partition_name is not None:
        all_in_names.append(partition_name)

    def _body(*args):
        operands = list(args)
        if partition_name is not None:
            operands.append(bass2jax.partition_id_tensor())
        outs = bass2jax._bass_exec_p.bind(
            *operands,
            out_avals=tuple(out_avals),
            in_names=tuple(all_in_names),
            out_names=tuple(out_names),
            lowering_input_output_aliases=(),
            sim_require_finite=True,
            sim_require_nnan=True,
            nc=nc,
        )
        return tuple(outs)

    n_outs = len(out_avals)
    donate = tuple(range(n_params, n_params + n_outs))
    devices = jax.devices()[:8]
    mesh = Mesh(np.asarray(devices), ("core",))
    in_specs = (PartitionSpec("core"),) * (n_params + n_outs)
    out_specs = (PartitionSpec("core"),) * n_outs
    sharded = jax.jit(
        shard_map(_body, mesh=mesh, in_specs=in_specs, out_specs=out_specs,
                  check_rep=False),
        donate_argnums=donate, keep_unused=True,
    )
    concat_in = [
        np.concatenate([np.asarray(in_maps[c][nm]) for c in range(8)], axis=0)
        for nm in in_names
    ]
    sh = NamedSharding(mesh, PartitionSpec("core"))
    dev_in = [jax.device_put(x, sh) for x in concat_in]

    def fresh_zeros():
        return [
            jax.device_put(np.zeros((8 * z.shape[0], *z.shape[1:]), z.dtype), sh)
            for z in zero_outs
        ]

    out = sharded(*dev_in, *fresh_zeros())
    jax.block_until_ready(out)

    def run_chain(k):
        zsets = [fresh_zeros() for _ in range(k)]
        for zs in zsets:
            jax.block_until_ready(zs)
        t0 = time.perf_counter()
        outs = [sharded(*dev_in, *zs) for zs in zsets]
        jax.block_until_ready(outs)
        return time.perf_counter() - t0

    run_chain(2)  # extra warmup
    slopes = []
    for _ in range(max(3, iters // 4)):
        t_a = run_chain(1)
        t_b = run_chain(chain)
        slopes.append((t_b - t_a) / (chain - 1.0))
    slopes.sort()
    med = slopes[len(slopes) // 2]
    print(f"bench(reps={reps}): slopes(us) = "
          f"{[f'{s*1e6:.0f}' for s in slopes]} -> median {med*1e6:.0f}us "
          f"min {slopes[0]*1e6:.0f}us")
    return med * 1e9


def bench_paired(inputs, pairs=10, hi_reps=8):
    """Paired-difference timing: interleave isolated calls of the reps=1 and
    reps=hi NEFFs; median of (t_hi - t_lo)/(hi-1) cancels slow drift."""
    import time

    r_lo = _BenchRunner(inputs, reps=1)
    r_hi = _BenchRunner(inputs, reps=hi_reps)
    r_lo.run(); r_hi.run(); r_lo.run(); r_hi.run()  # warm both
    diffs = []
    for _ in range(pairs):
        t_lo = r_lo.run()
        t_hi = r_hi.run()
        diffs.append((t_hi - t_lo) / (hi_reps - 1.0))
    diffs.sort()
    med = diffs[len(diffs) // 2]
    print(f"bench_paired: per-body diffs(us) = "
          f"{[f'{d*1e6:.0f}' for d in diffs]} -> median {med*1e6:.0f}us")
    return med * 1e9


class _BenchRunner:
    def __init__(self, inputs, reps):
        import jax
        from jax.experimental.shard_map import shard_map
        from jax.sharding import Mesh, NamedSharding, PartitionSpec
        from concourse import bass2jax, mybir as _mybir

        bass2jax.install_neuronx_cc_hook()
        nc = _get_program(reps)
        in_maps = _make_in_maps(**inputs)
        partition_name = (nc.partition_id_tensor.name
                          if nc.partition_id_tensor else None)
        in_names, out_names, out_avals, zero_outs = [], [], [], []
        for alloc in nc.m.functions[0].allocations:
            if not isinstance(alloc, _mybir.MemoryLocationSet):
                continue
            name = alloc.memorylocations[0].name
            if alloc.kind == "ExternalInput":
                if name != partition_name:
                    in_names.append(name)
            elif alloc.kind == "ExternalOutput":
                shape = tuple(alloc.tensor_shape)
                dtype = _mybir.dt.np(alloc.dtype)
                out_names.append(name)
                out_avals.append(jax.core.ShapedArray(shape, dtype))
                zero_outs.append(np.zeros(shape, dtype))
        n_params = len(in_names)
        all_in = list(in_names) + list(out_names)
        if partition_name is not None:
            all_in.append(partition_name)

        def _body(*args):
            operands = list(args)
            if partition_name is not None:
                operands.append(bass2jax.partition_id_tensor())
            return tuple(bass2jax._bass_exec_p.bind(
                *operands, out_avals=tuple(out_avals), in_names=tuple(all_in),
                out_names=tuple(out_names), lowering_input_output_aliases=(),
                sim_require_finite=True, sim_require_nnan=True, nc=nc))

        donate = tuple(range(n_params, n_params + len(out_avals)))
        devices = jax.devices()[:8]
        mesh = Mesh(np.asarray(devices), ("core",))
        specs = (PartitionSpec("core"),)
        self._sharded = jax.jit(
            shard_map(_body, mesh=mesh,
                      in_specs=specs * (n_params + len(out_avals)),
                      out_specs=specs * len(out_avals), check_rep=False),
            donate_argnums=donate, keep_unused=True)
        sh = NamedSharding(mesh, PartitionSpec("core"))
        self._dev_in = [jax.device_put(
            np.concatenate([np.asarray(in_maps[c][nm]) for c in range(8)],
                           axis=0), sh)
            for nm in in_names]
        self._zero_outs = zero_outs
        self._sh = sh
        self._jax = jax

    def run(self):
        import time
        jax = self._jax
        zs = [jax.device_put(
            np.zeros((8 * z.shape[0], *z.shape[1:]), z.dtype), self._sh)
            for z in self._zero_outs]
        jax.block_until_ready(zs)
        t0 = time.perf_counter()
        out = self._sharded(*self._dev_in, *zs)
        jax.block_until_ready(out)
        return time.perf_counter() - t0
